# revision 1
# baseline (speedup 1.0000x reference)
"""Trainium2 Bass kernel for NodeReadout: out = relu(concat([node_feature, segment_sum(edge_state, edge_dst)]) @ W + b).

Strategy (8 NeuronCores, no collectives):
  - Shard edges by DESTINATION OWNER: core c owns nodes [c*12500, (c+1)*12500)
    and receives exactly the edges destined to its nodes.
  - Host-side sharding lays each core's edge_state out in padded-CSR order
    (edges grouped by destination node, nodes grouped by padded degree,
    features transposed so SBUF partitions = feature dims). Each node's edge
    list is split into two halves mapped to partition ranges [0:64) / [64:128)
    so the DVE segment-reduction uses all 128 lanes.
  - Device: per degree-group strided-AP tensor_reduce (DVE) produces the
    segment sums; a 3-matmul PSUM accumulation (W1.T@nf + W2.T@aggLo +
    W2.T@aggHi) plus fused bias+ReLU on the scalar engine produces the output.
  - All 8 cores run the same NEFF with identical shapes (group structure is
    the per-degree max across cores; shortfall padded with zero rows / dummy
    node slots whose outputs are discarded on unshard).
"""

import math
import os
import sys
import types

import numpy as np

for _p in (
    "/root/.axon_site",
    "/root/.axon_site/_ro/trn_rl_repo",
    "/opt/trn_rl_repo",
):
    if os.path.isdir(_p) and _p not in sys.path:
        sys.path.append(_p)

N_CORES = 8
D = 64
SLAB = 512  # dense slab width (one PSUM bank of fp32)
CHUNK_ELEMS = int(os.environ.get("GNN_CHUNK", "4096"))
EBUF_BUFS = int(os.environ.get("GNN_EBUFS", "6"))


def _chunk_plan(groups, NSLOT, E2):
    """Pack the contiguous edge_t stream into uniform DMA chunks independent
    of group structure. Each chunk = (elem_off, n_elems, segs); each seg =
    (local_elem_off, h, n_nodes, slab_idx, slab_local_col) is one reduce.
    Chunk boundaries always fall on node boundaries."""
    segs = []  # (abs_elem_off, h, n_nodes, abs_col) clipped to slab bounds
    for d, ng, s_off, e_off in groups:
        h = d // 2
        s = 0
        while s < ng:
            col = s_off + s
            cn = min(ng - s, SLAB - col % SLAB)
            segs.append((e_off + s * h, h, cn, col))
            s += cn
    plan = []
    cur_eo, cur_fe, cur_segs = None, 0, []
    for eo, h, cn, col in segs:
        s = 0
        while s < cn:
            if cur_eo is None:
                cur_eo, cur_fe, cur_segs = eo + s * h, 0, []
            take = min(cn - s, (CHUNK_ELEMS - cur_fe) // h)
            if take == 0:
                plan.append((cur_eo, cur_fe, cur_segs))
                cur_eo, cur_fe, cur_segs = None, 0, []
                continue
            cur_segs.append(
                (cur_fe, h, take, (col + s) // SLAB, (col + s) % SLAB)
            )
            cur_fe += take * h
            s += take
            if cur_fe > CHUNK_ELEMS - 1:
                plan.append((cur_eo, cur_fe, cur_segs))
                cur_eo, cur_fe, cur_segs = None, 0, []
    if cur_segs:
        plan.append((cur_eo, cur_fe, cur_segs))
    assert sum(fe for _, fe, _ in plan) == E2
    return plan

_last_exec_time_ns = None
_last_results = None


def _install_shims():
    """Environment fixes: antenv.axon_hooks shim (NTFF profiling), no-op
    artifact upload, and a TileContext drain patch (this container's walrus
    rejects >1 sync-wait per instruction)."""
    # -- antenv.axon_hooks shim ------------------------------------------
    try:
        import antenv.axon_hooks  # noqa: F401
    except ImportError:
        try:
            import antenv

            mod = types.ModuleType("antenv.axon_hooks")
            mod._hook = None

            def set_axon_ntff_profile_hook(h):
                mod._hook = h

            def get_axon_ntff_profile_hook():
                return mod._hook

            mod.set_axon_ntff_profile_hook = set_axon_ntff_profile_hook
            mod.get_axon_ntff_profile_hook = get_axon_ntff_profile_hook
            sys.modules["antenv.axon_hooks"] = mod
            antenv.axon_hooks = mod
            try:
                from trn_agent_boot.trn_boot import _ntff_profile_via_ctypes

                so = "/opt/axon/libaxon_pjrt.so"
                if os.path.exists(so):
                    set_axon_ntff_profile_hook(_ntff_profile_via_ctypes(so))
            except Exception:
                pass
        except Exception:
            pass
    # -- artifact upload (needs a cloud bucket; not available here) ------
    try:
        import concourse.bass_utils as bu

        bu.upload_artifacts = lambda tmpdir: "local://" + tmpdir
    except Exception:
        pass
    # -- TileContext drain: split multi-sem waits ------------------------
    import concourse.mybir as mybir
    import concourse.tile as tile_mod
    from concourse.vector_clock import ScopedClock

    if getattr(tile_mod.TileContext, "_drain_patched", False):
        return
    tile_mod.TileContext._orig_drain_and_barrier = (
        tile_mod.TileContext._drain_and_barrier
    )

    def _drain_and_barrier(self, tick_clock, wait_clock):
        nc = self.nc
        probe = nc.sync.nop(nofuse=True, hint="drain_wait_split")
        wait_clock.add_sem_waits(
            probe.ins, ScopedClock({None: tick_clock.global_clock})
        )
        waits = list(probe.ins.sync_info.on_wait)
        probe.ins.sync_info.on_wait = waits[:1]
        for w in waits[1:]:
            nop = nc.sync.nop(nofuse=True, hint="drain_wait_split")
            nop.ins.sync_info = mybir.SyncInfo(on_update=[], on_wait=[w])
        nc.sync.drain()
        nc.all_engine_barrier()
        assert self.sems is not None
        popped = nc._tile_sem_poison_stack.pop()
        assert popped is self._sem_poison
        nc.clear_and_free_semaphores(list(self.sems.allocated().values()))
        nc.all_engine_barrier()

    tile_mod.TileContext._drain_and_barrier = _drain_and_barrier
    tile_mod.TileContext._patched_drain_and_barrier = _drain_and_barrier
    tile_mod.TileContext._drain_patched = True


def _split_multiwaits(nc):
    """Walrus here allows at most ONE sync-wait per instruction: hoist extra
    waits onto preceding NoOps on the same engine."""
    import concourse.mybir as mybir

    for fn in nc.m.functions:
        for blk in fn.blocks:
            insts = blk.instructions
            new = []
            for ins in insts:
                si = getattr(ins, "sync_info", None)
                waits = list(si.on_wait) if si is not None and si.on_wait else []
                if len(waits) > 1:
                    for j, w in enumerate(waits[:-1]):
                        nop = mybir.InstNoOp(
                            name=f"{ins.name}-wsplit{j}",
                            engine=ins.engine,
                            bass_nofuse=True,
                            sync_info=mybir.SyncInfo(on_update=[], on_wait=[w]),
                        )
                        new.append(nop)
                    si.on_wait = [waits[-1]]
                new.append(ins)
            blk.instructions[:] = new


def _prepare(node_feature, edge_state, edge_dst, W, b):
    """Host-side shard + layout. Returns (in_maps, groups, NSLOT, E2, col_node)."""
    node_feature = np.ascontiguousarray(np.asarray(node_feature), dtype=np.float32)
    edge_state = np.ascontiguousarray(np.asarray(edge_state), dtype=np.float32)
    edge_dst = np.asarray(edge_dst).astype(np.int64)
    W = np.ascontiguousarray(np.asarray(W), dtype=np.float32)
    b = np.asarray(b, dtype=np.float32).reshape(D, 1)

    N = node_feature.shape[0]
    # Global CSR: edges grouped by destination node.
    eid_sorted = np.argsort(edge_dst, kind="stable")
    deg = np.bincount(edge_dst, minlength=N)
    starts = np.cumsum(deg) - deg
    degp = np.maximum(2, ((deg + 1) // 2) * 2)

    # Degree-balanced sharding: nodes sorted by padded degree are dealt
    # round-robin to cores, so per-core degree histograms match to within 1
    # and the common group structure carries almost no cross-core padding.
    rank = np.argsort(degp, kind="stable")  # node ids in degree order
    # per-core node lists, in degree order
    core_nodes = [rank[c::N_CORES] for c in range(N_CORES)]

    all_degs = sorted(int(v) for v in np.unique(degp))
    counts = {d: int(np.count_nonzero(degp == d)) for d in all_degs}
    groups = []  # (deg, n_nodes_per_core, slot_off, elem_off_per_half)
    s_off = 0
    e_off = 0
    for d in all_degs:
        n = (counts[d] + N_CORES - 1) // N_CORES
        groups.append((d, n, s_off, e_off))
        s_off += n
        e_off += n * (d // 2)
    NSLOT = s_off
    E2 = e_off

    in_maps = []
    col_node = np.full((N_CORES, NSLOT), -1, dtype=np.int64)
    for c in range(N_CORES):
        nodes = core_nodes[c]  # global ids, ascending degp
        ndeg = degp[nodes]
        gidx = np.full((2, E2), -1, dtype=np.int64)
        for d, n, so, eo in groups:
            nodes_d = nodes[ndeg == d]
            k = len(nodes_d)
            if k == 0:
                continue
            h = d // 2
            col = starts[nodes_d][:, None] + np.arange(d)[None, :]
            valid = np.arange(d)[None, :] < deg[nodes_d][:, None]
            em = np.where(valid, eid_sorted[np.where(valid, col, 0)], -1)
            em = em.reshape(k, 2, h)
            gidx[0, eo : eo + k * h] = em[:, 0, :].ravel()
            gidx[1, eo : eo + k * h] = em[:, 1, :].ravel()
            col_node[c, so : so + k] = nodes_d
        X = np.zeros((2, E2, D), dtype=np.float32)
        for half in range(2):
            m = gidx[half] >= 0
            X[half, m] = edge_state[gidx[half, m]]
        edge_t = np.ascontiguousarray(
            X.transpose(0, 2, 1).reshape(2 * D, E2)
        )  # partitions [0:64)=half0 feats, [64:128)=half1 feats
        nf_t = np.zeros((D, NSLOT), dtype=np.float32)
        vm = col_node[c] >= 0
        nf_t[:, vm] = node_feature[col_node[c][vm]].T
        in_maps.append(
            {"edge_t": edge_t, "nf_t": nf_t, "W": W, "b": b}
        )
    return in_maps, groups, NSLOT, E2, col_node, N


def _build(groups, NSLOT, E2, for_sim=False):
    import concourse.bass as bass
    import concourse.mybir as mybir
    import concourse.tile as tile_mod
    from concourse.tile import TileContext

    if for_sim:
        # CoreSim can't digest the walrus single-wait workarounds; build
        # with the stock drain and skip the multi-wait split.
        tile_mod.TileContext._drain_and_barrier = (
            tile_mod.TileContext._orig_drain_and_barrier
        )

    f32 = mybir.dt.float32
    nc = bass.Bass("TRN2", target_bir_lowering=False, debug=False)
    edge_t = nc.declare_dram_parameter("edge_t", [128, E2], f32, isOutput=False)
    nf_t = nc.declare_dram_parameter("nf_t", [64, NSLOT], f32, isOutput=False)
    Wp = nc.declare_dram_parameter("W", [128, D], f32, isOutput=False)
    bp = nc.declare_dram_parameter("b", [64, 1], f32, isOutput=False)
    out_t = nc.declare_dram_parameter("out_t", [64, NSLOT], f32, isOutput=True)

    with TileContext(nc) as tc:
        with (
            tc.tile_pool(name="const", bufs=1) as cpool,
            tc.tile_pool(name="big", bufs=1) as bigpool,
            tc.tile_pool(name="edges", bufs=EBUF_BUFS) as epool,
            tc.tile_pool(name="psum", bufs=4, space="PSUM") as ppool,
            tc.tile_pool(name="outs", bufs=3) as opool,
        ):
            # Matmul operands must sit at base partition 0 on this HW, so:
            # m1: lhsT=W1 [64,64], rhs=nf [64,:]; m2: lhsT=[W2;W2] [128,64],
            # rhs=agg [128,:] (sums both halves in one K=128 matmul).
            w1 = cpool.tile([64, D], f32)
            nc.scalar.dma_start(out=w1[:], in_=Wp[0:64, :])
            w22 = cpool.tile([128, D], f32)
            nc.scalar.dma_start(out=w22[0:64, :], in_=Wp[64:128, :])
            nc.scalar.dma_start(out=w22[64:128, :], in_=Wp[64:128, :])
            bt = cpool.tile([64, 1], f32)
            nc.scalar.dma_start(out=bt[:], in_=bp[:])

            # Per-slab agg tiles: a dense slab depends only on the reduces
            # that wrote its own tile, so matmul/ACT/out-DMA interleave with
            # the aggregation stream instead of serializing at the end.
            n_slab = (NSLOT + SLAB - 1) // SLAB
            aggs = [
                bigpool.tile([128, SLAB], f32, name=f"agg{i}", tag=f"agg{i}")
                for i in range(n_slab)
            ]
            def dense_slab(sl):
                s = sl * SLAB
                n = min(SLAB, NSLOT - s)
                nfs = opool.tile([64, SLAB], f32, tag="nfs", name=f"nfs{sl}")
                nc.gpsimd.dma_start(out=nfs[:, :n], in_=nf_t[:, s : s + n])
                ps = ppool.tile(
                    [64, SLAB], f32, space="PSUM", tag="ps", name=f"ps{sl}"
                )
                nc.tensor.matmul(
                    out=ps[:, :n],
                    lhsT=w1[:],
                    rhs=nfs[:, :n],
                    start=True,
                    stop=False,
                )
                nc.tensor.matmul(
                    out=ps[:, :n],
                    lhsT=w22[:],
                    rhs=aggs[sl][:, :n],
                    start=False,
                    stop=True,
                )
                ob = opool.tile([64, SLAB], f32, tag="ob", name=f"ob{sl}")
                nc.scalar.activation(
                    out=ob[:, :n],
                    in_=ps[:, :n],
                    func=mybir.ActivationFunctionType.Relu,
                    bias=bt[:],
                )
                nc.gpsimd.dma_start(out=out_t[:, s : s + n], in_=ob[:, :n])

            # Uniform-size DMA chunks over the contiguous edge stream; the
            # per-group/per-slab reduce segments read from within the chunk.
            # Dense work for a slab is emitted right after the chunk that
            # completes it, so PE/ACT/out-DMA trail the stream closely.
            plan = _chunk_plan(groups, NSLOT, E2)
            last_chunk_of_slab = {}
            for ci, (_, _, segs) in enumerate(plan):
                for _, _, _, sl, _ in segs:
                    last_chunk_of_slab[sl] = ci
            for ci, (eo, fe, segs) in enumerate(plan):
                ebuf = epool.tile([128, CHUNK_ELEMS], f32, tag="ebuf")
                dma_eng = nc.sync if ci % 2 == 0 else nc.scalar
                dma_eng.dma_start(out=ebuf[:, :fe], in_=edge_t[:, eo : eo + fe])
                for loff, h, cn, sl, lc in segs:
                    nc.vector.tensor_reduce(
                        out=aggs[sl][:, lc : lc + cn],
                        in_=ebuf[:, loff : loff + cn * h].rearrange(
                            "p (n k) -> p n k", k=h
                        ),
                        axis=mybir.AxisListType.X,
                        op=mybir.AluOpType.add,
                    )
                for sl in sorted(
                    s for s, lc in last_chunk_of_slab.items() if lc == ci
                ):
                    dense_slab(sl)
    if for_sim:
        # restore the patched drain for subsequent HW builds
        tile_mod.TileContext._drain_and_barrier = (
            tile_mod.TileContext._patched_drain_and_barrier
        )
    else:
        _split_multiwaits(nc)
    return nc


def kernel(node_feature, edge_state, edge_dst, W, b):
    global _last_exec_time_ns, _last_results
    _install_shims()
    from concourse.bass_utils import run_bass_kernel_spmd

    in_maps, groups, NSLOT, E2, col_node, N = _prepare(
        node_feature, edge_state, edge_dst, W, b
    )
    nc = _build(groups, NSLOT, E2)
    trace = bool(os.environ.get("GNN_TRACE"))
    res = run_bass_kernel_spmd(
        nc, in_maps, core_ids=list(range(N_CORES)), trace=trace
    )
    _last_exec_time_ns = res.exec_time_ns
    _last_results = res
    out = np.zeros((N, D), dtype=np.float32)
    for c in range(N_CORES):
        ot = np.asarray(res.results[c]["out_t"])
        vm = col_node[c] >= 0
        out[col_node[c][vm]] = ot[:, vm].T
    return out


def last_exec_time_ns():
    return _last_exec_time_ns


def last_results():
    return _last_results



# revision 2
# speedup vs baseline: 1.7566x; 1.7566x over previous
"""Trainium2 Bass kernel for NodeReadout: out = relu(concat([node_feature, segment_sum(edge_state, edge_dst)]) @ W + b).

Strategy (8 NeuronCores, no collectives):
  - Shard edges by DESTINATION OWNER: core c owns ~12.5k nodes (degree-
    balanced round-robin), and receives exactly the edges destined to its
    nodes. All 8 cores run one NEFF with identical shapes.
  - Host lays each core's edge features out bf16, padded-CSR, transposed:
    SBUF partitions 0:64 = features of a node's first-half edges, 64:128 =
    second-half. Within a uniform-degree sub-chunk the columns are NODE-MINOR
    ([h pair-slot blocks] x [n nodes]), so the device segment-sum is a fold
    tree of full-width packed tensor_tensor adds (bf16 -> 2x DVE mode; a
    tensor_reduce would run at 1 col/cycle with no perf mode).
  - Device: stream edge chunks (DMA alternating sync/scalar queues), fold
    each sub-chunk down to 2 blocks, final fold writes the [128, SLAB] agg
    slab; per slab a 2-matmul PSUM accumulation (W1.T@nf + [W2;W2].T@agg,
    all bf16) plus fused bias+ReLU on the scalar engine produces bf16 out.
  - node_feature is one prefetched bf16 tile; output is flushed in slab
    bands on the gpsimd queue. Per-core HBM traffic ~30MB (vs 58MB fp32).
"""

import os
import sys
import types

import numpy as np
import ml_dtypes

for _p in (
    "/root/.axon_site",
    "/root/.axon_site/_ro/trn_rl_repo",
    "/opt/trn_rl_repo",
):
    if os.path.isdir(_p) and _p not in sys.path:
        sys.path.append(_p)

N_CORES = 8
D = 64
SLAB = 512  # dense slab width (one PSUM bank of fp32)
CHUNK = int(os.environ.get("GNN_CHUNK", "8192"))  # edge-stream cols per DMA
EBUF_BUFS = int(os.environ.get("GNN_EBUFS", "4"))
OUT_BAND = int(os.environ.get("GNN_OBAND", "6"))  # slabs per output DMA

BF16 = ml_dtypes.bfloat16

_last_exec_time_ns = None
_last_results = None


def _install_shims():
    """Environment fixes: antenv.axon_hooks shim (NTFF profiling), no-op
    artifact upload, and a TileContext drain patch (this container's walrus
    rejects >1 sync-wait per instruction)."""
    # -- antenv.axon_hooks shim ------------------------------------------
    try:
        import antenv.axon_hooks  # noqa: F401
    except ImportError:
        try:
            import antenv

            mod = types.ModuleType("antenv.axon_hooks")
            mod._hook = None

            def set_axon_ntff_profile_hook(h):
                mod._hook = h

            def get_axon_ntff_profile_hook():
                return mod._hook

            mod.set_axon_ntff_profile_hook = set_axon_ntff_profile_hook
            mod.get_axon_ntff_profile_hook = get_axon_ntff_profile_hook
            sys.modules["antenv.axon_hooks"] = mod
            antenv.axon_hooks = mod
            try:
                from trn_agent_boot.trn_boot import _ntff_profile_via_ctypes

                so = "/opt/axon/libaxon_pjrt.so"
                if os.path.exists(so):
                    set_axon_ntff_profile_hook(_ntff_profile_via_ctypes(so))
            except Exception:
                pass
        except Exception:
            pass
    # -- artifact upload (needs a cloud bucket; not available here) ------
    try:
        import concourse.bass_utils as bu

        bu.upload_artifacts = lambda tmpdir: "local://" + tmpdir
    except Exception:
        pass
    # -- TileContext drain: split multi-sem waits ------------------------
    import concourse.mybir as mybir
    import concourse.tile as tile_mod
    from concourse.vector_clock import ScopedClock

    if getattr(tile_mod.TileContext, "_drain_patched", False):
        return
    tile_mod.TileContext._orig_drain_and_barrier = (
        tile_mod.TileContext._drain_and_barrier
    )

    def _drain_and_barrier(self, tick_clock, wait_clock):
        nc = self.nc
        probe = nc.sync.nop(nofuse=True, hint="drain_wait_split")
        wait_clock.add_sem_waits(
            probe.ins, ScopedClock({None: tick_clock.global_clock})
        )
        waits = list(probe.ins.sync_info.on_wait)
        probe.ins.sync_info.on_wait = waits[:1]
        for w in waits[1:]:
            nop = nc.sync.nop(nofuse=True, hint="drain_wait_split")
            nop.ins.sync_info = mybir.SyncInfo(on_update=[], on_wait=[w])
        nc.sync.drain()
        nc.all_engine_barrier()
        assert self.sems is not None
        popped = nc._tile_sem_poison_stack.pop()
        assert popped is self._sem_poison
        nc.clear_and_free_semaphores(list(self.sems.allocated().values()))
        nc.all_engine_barrier()

    tile_mod.TileContext._drain_and_barrier = _drain_and_barrier
    tile_mod.TileContext._patched_drain_and_barrier = _drain_and_barrier
    tile_mod.TileContext._drain_patched = True


def _split_multiwaits(nc):
    """Walrus here allows at most ONE sync-wait per instruction: hoist extra
    waits onto preceding NoOps on the same engine."""
    import concourse.mybir as mybir

    for fn in nc.m.functions:
        for blk in fn.blocks:
            insts = blk.instructions
            new = []
            for ins in insts:
                si = getattr(ins, "sync_info", None)
                waits = list(si.on_wait) if si is not None and si.on_wait else []
                if len(waits) > 1:
                    for j, w in enumerate(waits[:-1]):
                        nop = mybir.InstNoOp(
                            name=f"{ins.name}-wsplit{j}",
                            engine=ins.engine,
                            bass_nofuse=True,
                            sync_info=mybir.SyncInfo(on_update=[], on_wait=[w]),
                        )
                        new.append(nop)
                    si.on_wait = [waits[-1]]
                new.append(ins)
            blk.instructions[:] = new


def _plan(groups):
    """Sub-chunks (uniform-degree node-minor blocks) packed into DMA chunks.
    sub = (col_off, h, n_nodes, slot0); chunk = (col_off, n_cols, [subs])."""
    subs = []
    for d, n, so, co in groups:
        h = d // 2
        nmax = max(1, CHUNK // h)
        i = 0
        while i < n:
            take = min(nmax, n - i)
            subs.append((co + i * h, h, take, so + i))
            i += take
    chunks = []
    cur, cur_off, cur_cols = None, 0, 0
    for sub in subs:
        sco, sh, sn, _ = sub
        w = sh * sn
        if cur is None or cur_cols + w > CHUNK:
            if cur is not None:
                chunks.append((cur_off, cur_cols, cur))
            cur, cur_off, cur_cols = [], sco, 0
        cur.append(sub)
        cur_cols += w
    if cur:
        chunks.append((cur_off, cur_cols, cur))
    return chunks


def _prepare(node_feature, edge_state, edge_dst, W, b):
    """Host-side shard + bf16 layout. Returns (in_maps, groups, chunks,
    NSLOT, E2, col_node, N)."""
    node_feature = np.ascontiguousarray(np.asarray(node_feature), dtype=np.float32)
    edge_state = np.ascontiguousarray(np.asarray(edge_state), dtype=np.float32)
    edge_dst = np.asarray(edge_dst).astype(np.int64)
    W16 = np.ascontiguousarray(np.asarray(W, dtype=np.float32).astype(BF16))
    b = np.asarray(b, dtype=np.float32).reshape(D, 1)

    N = node_feature.shape[0]
    # Global CSR: edges grouped by destination node.
    eid_sorted = np.argsort(edge_dst, kind="stable")
    deg = np.bincount(edge_dst, minlength=N)
    starts = np.cumsum(deg) - deg
    degp = np.maximum(2, ((deg + 1) // 2) * 2)

    # Degree-balanced sharding: nodes sorted by padded degree are dealt
    # round-robin to cores, so per-core degree histograms match to within 1
    # and the common group structure carries almost no cross-core padding.
    rank = np.argsort(degp, kind="stable")
    core_nodes = [rank[c::N_CORES] for c in range(N_CORES)]

    all_degs = sorted(int(v) for v in np.unique(degp))
    counts = {d: int(np.count_nonzero(degp == d)) for d in all_degs}
    groups = []  # (deg, n_nodes_per_core, slot_off, col_off)
    s_off = 0
    c_off = 0
    for d in all_degs:
        n = (counts[d] + N_CORES - 1) // N_CORES
        groups.append((d, n, s_off, c_off))
        s_off += n
        c_off += n * (d // 2)
    NSLOT = s_off
    E2 = c_off
    chunks = _plan(groups)

    es16 = edge_state.astype(BF16)
    nf16 = node_feature.astype(BF16)
    subs_by_group = {}
    for ch in chunks:
        for sub in ch[2]:
            subs_by_group.setdefault(sub[0], sub)  # keyed by col_off

    in_maps = []
    col_node = np.full((N_CORES, NSLOT), -1, dtype=np.int64)
    for c in range(N_CORES):
        nodes = core_nodes[c]  # global ids, ascending degp
        ndeg = degp[nodes]
        edge_tc = np.zeros((2 * D, E2), dtype=BF16)
        for d, n, so, co in groups:
            h = d // 2
            nodes_d = nodes[ndeg == d]
            k = len(nodes_d)
            G = np.zeros((n, 2, h, D), dtype=BF16)
            if k:
                col = starts[nodes_d][:, None] + np.arange(d)[None, :]
                valid = np.arange(d)[None, :] < deg[nodes_d][:, None]
                em = np.where(valid, eid_sorted[np.where(valid, col, 0)], -1)
                mvalid = em >= 0
                Gk = np.zeros((k, d, D), dtype=BF16)
                Gk[mvalid] = es16[em[mvalid]]
                G[:k] = Gk.reshape(k, 2, h, D)
                col_node[c, so : so + k] = nodes_d
            # node-minor blocks per sub-chunk of this group
            i = 0
            nmax = max(1, CHUNK // h)
            while i < n:
                take = min(nmax, n - i)
                blk = G[i : i + take].transpose(1, 3, 2, 0).reshape(2 * D, h * take)
                edge_tc[:, co + i * h : co + (i + take) * h] = blk
                i += take
        nf_tc = np.zeros((D, NSLOT), dtype=BF16)
        vm = col_node[c] >= 0
        nf_tc[:, vm] = nf16[col_node[c][vm]].T
        in_maps.append({"edge_t": edge_tc, "nf_t": nf_tc, "W": W16, "b": b})
    return in_maps, groups, chunks, NSLOT, E2, col_node, N


def _build(groups, chunks, NSLOT, E2, for_sim=False):
    import concourse.bass as bass
    import concourse.mybir as mybir
    import concourse.tile as tile_mod
    from concourse.tile import TileContext

    if for_sim:
        # CoreSim can't digest the walrus single-wait workarounds; build
        # with the stock drain and skip the multi-wait split.
        tile_mod.TileContext._drain_and_barrier = (
            tile_mod.TileContext._orig_drain_and_barrier
        )

    f32 = mybir.dt.float32
    bf16 = mybir.dt.bfloat16
    nc = bass.Bass("TRN2", target_bir_lowering=False, debug=False)
    edge_t = nc.declare_dram_parameter("edge_t", [128, E2], bf16, isOutput=False)
    nf_t = nc.declare_dram_parameter("nf_t", [64, NSLOT], bf16, isOutput=False)
    Wp = nc.declare_dram_parameter("W", [128, D], bf16, isOutput=False)
    bp = nc.declare_dram_parameter("b", [64, 1], f32, isOutput=False)
    out_t = nc.declare_dram_parameter("out_t", [64, NSLOT], bf16, isOutput=True)

    n_slab = (NSLOT + SLAB - 1) // SLAB
    add = mybir.AluOpType.add

    with TileContext(nc) as tc:
        with (
            tc.tile_pool(name="const", bufs=1) as cpool,
            tc.tile_pool(name="big", bufs=1) as bigpool,
            tc.tile_pool(name="edges", bufs=EBUF_BUFS) as epool,
            tc.tile_pool(name="psum", bufs=4, space="PSUM") as ppool,
        ):
            # Matmul operands must sit at base partition 0 on this HW, so:
            # m1: lhsT=W1 [64,64], rhs=nf [64,:]; m2: lhsT=[W2;W2] [128,64],
            # rhs=agg [128,:] (sums both halves in one K=128 matmul).
            w1 = cpool.tile([64, D], bf16)
            nc.scalar.dma_start(out=w1[:], in_=Wp[0:64, :])
            w22 = cpool.tile([128, D], bf16)
            nc.scalar.dma_start(out=w22[0:64, :], in_=Wp[64:128, :])
            nc.scalar.dma_start(out=w22[64:128, :], in_=Wp[64:128, :])
            bt = cpool.tile([64, 1], f32)
            nc.scalar.dma_start(out=bt[:], in_=bp[:])
            nfb = bigpool.tile([64, NSLOT], bf16, name="nfb")
            nc.gpsimd.dma_start(out=nfb[:], in_=nf_t[:])
            outb = bigpool.tile([64, NSLOT], bf16, name="outb")

            aggs = [
                bigpool.tile([128, SLAB], bf16, name=f"agg{i}", tag=f"agg{i}")
                for i in range(n_slab)
            ]

            def dense_slab(sl):
                s = sl * SLAB
                n = min(SLAB, NSLOT - s)
                ps = ppool.tile(
                    [64, SLAB], f32, space="PSUM", tag="ps", name=f"ps{sl}"
                )
                nc.tensor.matmul(
                    out=ps[:, :n],
                    lhsT=w1[:],
                    rhs=nfb[:, s : s + n],
                    start=True,
                    stop=False,
                )
                nc.tensor.matmul(
                    out=ps[:, :n],
                    lhsT=w22[:],
                    rhs=aggs[sl][:, :n],
                    start=False,
                    stop=True,
                )
                nc.scalar.activation(
                    out=outb[:, s : s + n],
                    in_=ps[:, :n],
                    func=mybir.ActivationFunctionType.Relu,
                    bias=bt[:],
                )

            def emit_final(m, ebuf, lo, n, s0):
                # final fold (m==2) or copy (m==1, i.e. degree-2 group) into
                # the agg slab tiles, split at slab boundaries
                i = 0
                while i < n:
                    sl = (s0 + i) // SLAB
                    lc = (s0 + i) % SLAB
                    cw = min(n - i, SLAB - lc)
                    if m == 2:
                        nc.vector.tensor_tensor(
                            out=aggs[sl][:, lc : lc + cw],
                            in0=ebuf[:, lo + i : lo + i + cw],
                            in1=ebuf[:, lo + n + i : lo + n + i + cw],
                            op=add,
                        )
                    else:
                        nc.vector.tensor_copy(
                            out=aggs[sl][:, lc : lc + cw],
                            in_=ebuf[:, lo + i : lo + i + cw],
                        )
                    i += cw

            last_chunk_of_slab = {}
            for ci, (c_off, fe, csubs) in enumerate(chunks):
                for sco, sh, sn, ss0 in csubs:
                    for sl in range(ss0 // SLAB, (ss0 + sn - 1) // SLAB + 1):
                        last_chunk_of_slab[sl] = ci

            flushed = -1
            for ci, (c_off, fe, csubs) in enumerate(chunks):
                ebuf = epool.tile([128, CHUNK], bf16, tag="ebuf")
                dma_eng = nc.sync if ci % 2 == 0 else nc.scalar
                dma_eng.dma_start(out=ebuf[:, :fe], in_=edge_t[:, c_off : c_off + fe])
                for sco, sh, sn, ss0 in csubs:
                    lo = sco - c_off
                    m = sh
                    while m > 2:
                        k = m // 2
                        nc.vector.tensor_tensor(
                            out=ebuf[:, lo : lo + k * sn],
                            in0=ebuf[:, lo : lo + k * sn],
                            in1=ebuf[:, lo + (m - k) * sn : lo + m * sn],
                            op=add,
                        )
                        m -= k
                    emit_final(m, ebuf, lo, sn, ss0)
                for sl in sorted(
                    s for s, lc in last_chunk_of_slab.items() if lc == ci
                ):
                    dense_slab(sl)
                    if sl == n_slab - 1 or (sl - flushed) >= OUT_BAND:
                        a = (flushed + 1) * SLAB
                        bnd = min((sl + 1) * SLAB, NSLOT)
                        nc.gpsimd.dma_start(
                            out=out_t[:, a:bnd], in_=outb[:, a:bnd]
                        )
                        flushed = sl
    if for_sim:
        # restore the patched drain for subsequent HW builds
        tile_mod.TileContext._drain_and_barrier = (
            tile_mod.TileContext._patched_drain_and_barrier
        )
    else:
        _split_multiwaits(nc)
    return nc


def kernel(node_feature, edge_state, edge_dst, W, b):
    global _last_exec_time_ns, _last_results
    _install_shims()
    from concourse.bass_utils import run_bass_kernel_spmd

    in_maps, groups, chunks, NSLOT, E2, col_node, N = _prepare(
        node_feature, edge_state, edge_dst, W, b
    )
    nc = _build(groups, chunks, NSLOT, E2)
    trace = bool(os.environ.get("GNN_TRACE"))
    res = run_bass_kernel_spmd(
        nc, in_maps, core_ids=list(range(N_CORES)), trace=trace
    )
    _last_exec_time_ns = res.exec_time_ns
    _last_results = res
    out = np.zeros((N, D), dtype=np.float32)
    for c in range(N_CORES):
        ot = np.asarray(res.results[c]["out_t"]).astype(np.float32)
        vm = col_node[c] >= 0
        out[col_node[c][vm]] = ot[:, vm].T
    return out


def last_exec_time_ns():
    return _last_exec_time_ns


def last_results():
    return _last_results
